# revision 14
# baseline (speedup 1.0000x reference)
"""Trainium2 Bass kernel for nn_MessagePassingNN (gnn_message_passing).

B, N, F, H, A, T = 4, 256, 64, 256, 16, 3

Sharding: 8 cores = (batch b = c//2, receiver-half c%2). Node indexing is
core-relative ([my 128 | partner 128]); host permutes inputs per core.

v2 design (from HW trace analysis of the 437us baseline):
- e-loop work w[i,j,h] = hjbT[h,j] + 32*(adj[i,j]-1) (+hi in the accum op),
  relu + sum_j, is spread across FOUR engines:
    masks:  PE (identity-matmul hjbT into PSUM + K=1 outer of 32(adj-1)),
            GpSimd (batched TT add vs adj broadcast), DVE (same).
    accums: ScalarE activation(Relu, bias=hi, accum_out) reading PSUM (PE
            tiles) or SBUF (GPS/DVE tiles); DVE scalar_tensor_tensor.
- GRU is receiver-chunked (2 x 64) so the pair-exchange DMA starts early;
  Whh matmuls are hoisted to iteration start (PE idle during e-loop).
- h exchange in bf16 (partner half only feeds the hjbT matmul).
- Last iteration exchanges only the [H,1] graph-sum (readout), not h.
- DMA order: critical weights first, 8MB adjacency broadcast last.
"""

import sys

sys.path.insert(0, "/opt/trn_rl_repo")

import numpy as np

import concourse.bass as bass
import concourse.bacc as bacc
import concourse.tile as tile
from concourse import mybir
from concourse.bass_utils import run_bass_kernel_spmd

B, N, F, H, A, T = 4, 256, 64, 256, 16, 3
NLOC = 128          # receivers per core
HT = H // 128       # h-dim tiles (2)
NCH = 2             # receiver chunks per iteration
CW = NLOC // NCH    # chunk width (64)
f32 = mybir.dt.float32
bf16 = mybir.dt.bfloat16
BF16_NP = mybir.dt.np(bf16)

# e-loop engine split, per 64-receiver chunk (rest of masks go to DVE)
import os
MKPE = int(os.environ.get('MKPE', 26))           # receivers masked on PE (first in chunk)
MKGPS = int(os.environ.get('MKGPS', 32))          # receivers masked on GpSimd (next)
ACC_SC_PE = int(os.environ.get('ACC_SC_PE', 13))      # of the PE-masked receivers, how many accumulate on SC
ACC_SC_GPS = int(os.environ.get('ACC_SC_GPS', 13))     # of the GPS-masked receivers, how many accumulate on SC
IB = 4              # receivers per batched TT mask op

_CACHE = {}


class _WSb:
    """SBUF weight holder: W [K, M] stored as [128, (K//128)*M]."""

    def __init__(self, nc, pool, dram, K, M, name, dt=f32, eng=None):
        self.mcols = M
        self.kt = K // 128
        self.sb = pool.tile([128, self.kt * M], dt, name=name, tag=name)
        (eng or nc.sync).dma_start(out=self.sb[:], in_=dram[:])

    def __getitem__(self, sl):
        return self.sb[sl]


def build_program():
    nc = bacc.Bacc("TRN2", target_bir_lowering=False, debug=False, num_devices=8)

    # ---------------- I/O ----------------
    xT_d = nc.dram_tensor("xT", [F, N], f32, kind="ExternalInput")
    adj_d = nc.dram_tensor("adjb", [NLOC, N], bf16, kind="ExternalInput")
    NPE_ROWS = max(1, NCH * MKPE)
    adjpe_d = nc.dram_tensor("adjpe", [1, NPE_ROWS * N], bf16, kind="ExternalInput")
    deg_d = nc.dram_tensor("degr", [1, NLOC], f32, kind="ExternalInput")
    w_pre1 = nc.dram_tensor("pre_W1", [F, H], f32, kind="ExternalInput")
    w_pre2 = nc.dram_tensor("pre_W2", [128, HT * H], f32, kind="ExternalInput")
    w_m1i = nc.dram_tensor("W1ib", [128, HT * H], bf16, kind="ExternalInput")
    w_m1j = nc.dram_tensor("W1jb", [128, HT * H], bf16, kind="ExternalInput")
    w_m2 = nc.dram_tensor("W2m", [128, HT * H], f32, kind="ExternalInput")
    w_ih = nc.dram_tensor("Wih", [128, HT * 3 * H], f32, kind="ExternalInput")
    w_hh = nc.dram_tensor("Whh", [128, HT * 3 * H], f32, kind="ExternalInput")
    w_ro1 = nc.dram_tensor("roW1", [128, HT * H], f32, kind="ExternalInput")
    w_ro2 = nc.dram_tensor("roW2", [128, HT * A], f32, kind="ExternalInput")
    preb1_d = nc.dram_tensor("preb1c", [128, HT], f32, kind="ExternalInput")
    preb2_d = nc.dram_tensor("preb2c", [128, HT], f32, kind="ExternalInput")
    msgb1_d = nc.dram_tensor("msgb1c", [128, HT], f32, kind="ExternalInput")
    msgb2_d = nc.dram_tensor("msgb2r", [1, H], f32, kind="ExternalInput")
    brz_d = nc.dram_tensor("brzc", [128, 4], f32, kind="ExternalInput")
    bihn_d = nc.dram_tensor("bihnc", [128, HT], f32, kind="ExternalInput")
    bhhn_d = nc.dram_tensor("bhhnc", [128, HT], f32, kind="ExternalInput")
    rob1_d = nc.dram_tensor("rob1c", [128, HT], f32, kind="ExternalInput")
    rob2_d = nc.dram_tensor("rob2c", [A, 1], f32, kind="ExternalInput")
    ident_d = nc.dram_tensor("identb", [128, 128], bf16, kind="ExternalInput")
    ones_d = nc.dram_tensor("onesr", [1, 128], bf16, kind="ExternalInput")
    q_out = nc.dram_tensor("q_out", [A, 1], f32, kind="ExternalOutput")

    # collective bounce buffers: h exchange (bf16) for t<T-1, g exchange (f32)
    cc_in = [nc.dram_tensor(f"cc_in_{t}", [H, NLOC], bf16) for t in range(T - 1)]
    cc_out = [nc.dram_tensor(f"cc_out_{t}", [H, NLOC], bf16) for t in range(T - 1)]
    gcc_in = nc.dram_tensor("gcc_in", [H, 1], f32)
    gcc_out = nc.dram_tensor("gcc_out", [H, 1], f32)
    groups = [[0, 1], [2, 3], [4, 5], [6, 7]]

    with tile.TileContext(nc) as tc:
        import contextlib

        with contextlib.ExitStack() as ctx:
            singles = ctx.enter_context(tc.tile_pool(name="singles", bufs=1))
            work = ctx.enter_context(tc.tile_pool(name="work", bufs=3))
            eloop = ctx.enter_context(tc.tile_pool(name="eloop", bufs=6))
            psp = ctx.enter_context(tc.tile_pool(name="psp", bufs=2, space="PSUM"))
            psg = ctx.enter_context(tc.tile_pool(name="psg", bufs=4, space="PSUM"))
            psh = ctx.enter_context(tc.tile_pool(name="psh", bufs=1, space="PSUM"))

            # ------------- weights/constants to SBUF (critical first) -------
            xT_sb = work.tile([F, N], f32, name="xT_sb", tag="xT_sb")
            nc.sync.dma_start(out=xT_sb[:], in_=xT_d[:])
            Wpre1 = singles.tile([F, H], f32)
            nc.sync.dma_start(out=Wpre1[:], in_=w_pre1[:])
            preb1 = singles.tile([128, HT], f32, name="preb1", tag="preb1")
            nc.sync.dma_start(out=preb1[:], in_=preb1_d[:])
            preb2 = singles.tile([128, HT], f32, name="preb2", tag="preb2")
            nc.sync.dma_start(out=preb2[:], in_=preb2_d[:])
            msgb1 = singles.tile([128, HT], f32, name="msgb1", tag="msgb1")
            nc.sync.dma_start(out=msgb1[:], in_=msgb1_d[:])
            W_pre2 = _WSb(nc, singles, w_pre2[:], H, H, "Wpre2", f32)
            W_m1j = _WSb(nc, singles, w_m1j[:], H, H, "Wm1j", bf16)
            W_m1i = _WSb(nc, singles, w_m1i[:], H, H, "Wm1i", bf16)
            ident = singles.tile([128, 128], bf16, name="ident", tag="ident")
            nc.sync.dma_start(out=ident[:], in_=ident_d[:])
            ones_r = singles.tile([1, 128], bf16, name="ones_r", tag="ones_r")
            nc.sync.dma_start(out=ones_r[:], in_=ones_d[:])
            adj_pe = singles.tile([1, NPE_ROWS * N], bf16, name="adjpe", tag="adjpe")
            nc.gpsimd.dma_start(out=adj_pe[:], in_=adjpe_d[:])
            W_hh = _WSb(nc, singles, w_hh[:], H, 3 * H, "Whh", f32)
            W_m2 = _WSb(nc, singles, w_m2[:], H, H, "Wm2", f32, eng=nc.gpsimd)
            W_ih = _WSb(nc, singles, w_ih[:], H, 3 * H, "Wih", f32, eng=nc.gpsimd)

            def _load(shape, dram, name, eng=nc.sync):
                t_ = singles.tile(list(shape), f32, name=name, tag=name)
                eng.dma_start(out=t_[:], in_=dram[:])
                return t_

            msgb2 = _load([1, H], msgb2_d, "msgb2")
            brz = _load([128, 4], brz_d, "brz")
            bihn = _load([128, HT], bihn_d, "bihn")
            bhhn = _load([128, HT], bhhn_d, "bhhn")
            deg_row = _load([1, NLOC], deg_d, "degr")
            W_ro1 = _WSb(nc, singles, w_ro1[:], H, H, "Wro1", f32, eng=nc.gpsimd)
            W_ro2 = _WSb(nc, singles, w_ro2[:], H, A, "Wro2", f32, eng=nc.gpsimd)
            rob1 = _load([128, HT], rob1_d, "rob1", eng=nc.gpsimd)
            rob2 = singles.tile([A, 1], f32, name="rob2", tag="rob2")
            nc.gpsimd.dma_start(out=rob2[:], in_=rob2_d[:])

            zeros_e = singles.tile([128, N], bf16)
            nc.vector.memset(zeros_e[:], 0.0)

            # receiver -> (mask path, accum engine) map, per chunk pattern
            # mask: 0=PE 1=GPS 2=DVE ; acc: 0=SC 1=DVE
            rmap = []
            for r in range(CW):
                if r < MKPE:
                    rmap.append((0, 0 if r < ACC_SC_PE else 1))
                elif r < MKPE + MKGPS:
                    g = r - MKPE
                    rmap.append((1, 0 if g < ACC_SC_GPS else 1))
                else:
                    rmap.append((2, 1))

            # per-group adjacency broadcast tiles (GPS/DVE-masked rows only)
            tt_groups = []  # (chunk, [receivers]) with consecutive receivers
            for c in range(NCH):
                base = c * CW
                for path in (1, 2):
                    recs = [base + r for r in range(CW) if rmap[r][0] == path]
                    for k in range(0, len(recs), IB):
                        tt_groups.append(recs[k:k + IB])
            adj_bcg = {}
            for gi, grp in enumerate(tt_groups):
                r0, gl = grp[0], len(grp)
                tl = singles.tile([128, gl * N], bf16, name=f"adjg{r0}", tag=f"adjg{r0}")
                bc_in = bass.AP(
                    tensor=adj_d, offset=r0 * N,
                    ap=[[0, 128], [N, gl], [1, N]],
                )
                eng = nc.gpsimd if gi % 2 == 0 else nc.sync
                eng.dma_start(out=tl[:], in_=bc_in)
                adj_bcg[r0] = tl

            # ---------------- preprocess: h0 ----------------
            p1 = [work.tile([128, N], f32, name=f"p1_{ht}", tag=f"p1_{ht}") for ht in range(HT)]
            for ht in range(HT):
                ps = psp.tile([128, 512], f32, name="ps", tag="ps")
                nc.tensor.matmul(ps[:, 0:N], Wpre1[:, ht * 128:(ht + 1) * 128], xT_sb[:], start=True, stop=True)
                nc.scalar.activation(p1[ht][:], ps[:, 0:N], mybir.ActivationFunctionType.Relu, bias=preb1[:, ht:ht + 1])
            h0 = [work.tile([128, N], f32, name=f"h0_{ht}", tag=f"h0_{ht}") for ht in range(HT)]
            hTb = [singles.tile([128, N], bf16, name=f"hTb{ht}", tag=f"hTb{ht}") for ht in range(HT)]
            for ht in range(HT):
                ps = psp.tile([128, 512], f32, name="ps", tag="ps")
                for kt in range(HT):
                    nc.tensor.matmul(ps[:, 0:N], W_pre2[:, kt * H + ht * 128: kt * H + (ht + 1) * 128], p1[kt][:], start=(kt == 0), stop=(kt == HT - 1))
                nc.scalar.activation(h0[ht][:], ps[:, 0:N], mybir.ActivationFunctionType.Identity, bias=preb2[:, ht:ht + 1])
                nc.vector.tensor_copy(hTb[ht][:], h0[ht][:])

            pe_pos = {}
            for c in range(NCH):
                for r in range(MKPE):
                    pe_pos[c * CW + r] = len(pe_pos)

            # persistent local-h (f32) ping-pong buffers
            hbuf = [[singles.tile([128, NLOC], f32, name=f"hb{p}_{ht}", tag=f"hb{p}_{ht}")
                     for ht in range(HT)] for p in range(2)]
            for ht in range(HT):
                nc.vector.tensor_copy(hbuf[0][ht][:], h0[ht][:, 0:NLOC])

            # ---------------- message passing iterations ----------------
            for t in range(T):
                hloc = hbuf[t % 2]
                hnxt = hbuf[(t + 1) % 2]
                # hjbT = (h @ W1_j + b1).T  [h, j] bf16
                hjbT = [work.tile([128, N], bf16, name=f"hjbT{ht}", tag=f"hjbT{ht}") for ht in range(HT)]
                for ht in range(HT):
                    ps = psp.tile([128, 512], f32, name="ps", tag="ps")
                    for kt in range(HT):
                        nc.tensor.matmul(ps[:, 0:N], W_m1j[:, kt * H + ht * 128: kt * H + (ht + 1) * 128], hTb[kt][:], start=(kt == 0), stop=(kt == HT - 1))
                    nc.scalar.activation(hjbT[ht][:], ps[:, 0:N], mybir.ActivationFunctionType.Identity, bias=msgb1[:, ht:ht + 1])
                # hiT = (h_loc @ W1_i).T [h, i] f32
                hiTf = [work.tile([128, NLOC], f32, name=f"hiTf{ht}", tag=f"hiTf{ht}") for ht in range(HT)]
                for ht in range(HT):
                    ps = psp.tile([128, 512], f32, name="ps", tag="ps")
                    for kt in range(HT):
                        nc.tensor.matmul(ps[:, 0:NLOC], W_m1i[:, kt * H + ht * 128: kt * H + (ht + 1) * 128], hTb[kt][:, 0:NLOC], start=(kt == 0), stop=(kt == HT - 1))
                    nc.vector.tensor_copy(hiTf[ht][:], ps[:, 0:NLOC])

                # hoisted Whh matmuls (depend only on h)
                ps_rz = psh.tile([128, 512], f32, name="ps_rz", tag="ps_rz")
                ps_gh = psh.tile([128, 512], f32, name="ps_gh", tag="ps_gh")
                for mt in range(4):  # r0 r1 z0 z1
                    for kt in range(HT):
                        nc.tensor.matmul(ps_rz[:, mt * 128:(mt + 1) * 128], W_hh[:, kt * 768 + mt * 128: kt * 768 + (mt + 1) * 128], hloc[kt][:], start=(kt == 0), stop=False)
                for ht in range(HT):  # hn into ps_gh cols 256+
                    for kt in range(HT):
                        nc.tensor.matmul(ps_gh[:, 256 + ht * 128: 256 + (ht + 1) * 128], W_hh[:, kt * 768 + (4 + ht) * 128: kt * 768 + (5 + ht) * 128], hloc[kt][:], start=(kt == 0), stop=(kt == HT - 1))

                aggT = [work.tile([128, NLOC], f32, name=f"aggT{ht}", tag=f"aggT{ht}") for ht in range(HT)]
                rz_sb = work.tile([128, 512], f32, name="rz_sb", tag="rz_sb")

                for c in range(NCH):
                    base = c * CW
                    # ---- e-loop for this chunk ----
                    # emit PE groups (2 receivers x 2 ht per group)
                    pe_recs = [base + r for r in range(CW) if rmap[r][0] == 0]
                    gps_recs = [base + r for r in range(CW) if rmap[r][0] == 1]
                    dve_recs = [base + r for r in range(CW) if rmap[r][0] == 2]

                    def emit_accum(i, ht, src_ap, r):
                        scr = eloop.tile([128, N], bf16, name="scr", tag="scr")
                        if rmap[r][1] == 0:
                            nc.scalar.activation(
                                scr[:], src_ap, mybir.ActivationFunctionType.Relu,
                                bias=hiTf[ht][:, i:i + 1], accum_out=aggT[ht][:, i:i + 1])
                        else:
                            nc.vector.scalar_tensor_tensor(
                                out=scr[:], in0=src_ap, scalar=hiTf[ht][:, i:i + 1],
                                in1=zeros_e[:], op0=mybir.AluOpType.add,
                                op1=mybir.AluOpType.max, accum_out=aggT[ht][:, i:i + 1])

                    # interleave: PE pairs, GPS quads, DVE quads
                    pe_groups = [pe_recs[k:k + 2] for k in range(0, len(pe_recs), 2)]
                    gps_groups = [gps_recs[k:k + IB] for k in range(0, len(gps_recs), IB)]
                    dve_groups = [dve_recs[k:k + IB] for k in range(0, len(dve_recs), IB)]

                    def emit_pe_group(grp):
                        pg = [psg.tile([128, 512], f32, name="pg", tag="pg") for _ in range(HT)]
                        for ht in range(HT):
                            for k, i in enumerate(grp):
                                nc.tensor.matmul(pg[ht][:, k * 256:k * 256 + N], ident[:], hjbT[ht][:], start=True, stop=False)
                        for ht in range(HT):
                            for k, i in enumerate(grp):
                                nc.tensor.matmul(pg[ht][:, k * 256:k * 256 + N], ones_r[:], adj_pe[0:1, pe_pos[i] * N:(pe_pos[i] + 1) * N], start=False, stop=True)
                        for ht in range(HT):
                            for k, i in enumerate(grp):
                                emit_accum(i, ht, pg[ht][:, k * 256:k * 256 + N], i - base)

                    def emit_tt_group(grp, eng):
                        for ht in range(HT):
                            hjb_rep = bass.AP(
                                tensor=hjbT[ht].tensor, offset=hjbT[ht].offset,
                                ap=[hjbT[ht].ap[0], [0, len(grp)], [1, N]])
                            w = eloop.tile([128, IB * N], bf16, name="w", tag="w")
                            eng.tensor_tensor(
                                out=w[:, 0:len(grp) * N], in0=hjb_rep,
                                in1=adj_bcg[grp[0]][:],
                                op=mybir.AluOpType.add)
                            for k, i in enumerate(grp):
                                emit_accum(i, ht, w[:, k * N:(k + 1) * N], i - base)

                    npe, ngps, ndve = len(pe_groups), len(gps_groups), len(dve_groups)
                    nmax = max(npe, ngps, ndve)
                    for k in range(nmax):
                        if k < npe:
                            emit_pe_group(pe_groups[k])
                        if k < ngps:
                            emit_tt_group(gps_groups[k], nc.gpsimd)
                        if k < ndve:
                            emit_tt_group(dve_groups[k], nc.vector)

                    # ---- GRU for this chunk ----
                    C0, C1 = base, base + CW
                    ps_m = psp.tile([128, 512], f32, name="ps", tag="ps")
                    for ht in range(HT):
                        for kt in range(HT):
                            nc.tensor.matmul(ps_m[:, ht * CW:(ht + 1) * CW], W_m2[:, kt * H + ht * 128: kt * H + (ht + 1) * 128], aggT[kt][:, C0:C1], start=(kt == 0), stop=False)
                        nc.tensor.matmul(ps_m[:, ht * CW:(ht + 1) * CW], msgb2[0:1, ht * 128:(ht + 1) * 128], deg_row[0:1, C0:C1], start=False, stop=True)
                    msgTb = work.tile([128, 2 * CW], f32, name="msgTb", tag="msgTb")
                    for ht in range(HT):
                        nc.vector.tensor_copy(msgTb[:, ht * CW:(ht + 1) * CW], ps_m[:, ht * CW:(ht + 1) * CW])
                    # gate matmuls (Wih part)
                    for mt in range(4):
                        for kt in range(HT):
                            nc.tensor.matmul(ps_rz[:, mt * 128 + C0: mt * 128 + C1], W_ih[:, kt * 768 + mt * 128: kt * 768 + (mt + 1) * 128], msgTb[:, kt * CW:(kt + 1) * CW], start=False, stop=(kt == HT - 1))
                    for ht in range(HT):
                        for kt in range(HT):
                            nc.tensor.matmul(ps_gh[:, ht * 128 + C0: ht * 128 + C1], W_ih[:, kt * 768 + (4 + ht) * 128: kt * 768 + (5 + ht) * 128], msgTb[:, kt * CW:(kt + 1) * CW], start=(kt == 0), stop=(kt == HT - 1))
                    for mt in range(4):
                        nc.scalar.activation(rz_sb[:, mt * 128 + C0: mt * 128 + C1], ps_rz[:, mt * 128 + C0: mt * 128 + C1], mybir.ActivationFunctionType.Sigmoid, bias=brz[:, mt:mt + 1])
                    for ht in range(HT):
                        # rhn = (gh_n + bhhn) * r
                        rhn = work.tile([128, CW], f32, name="rhn", tag="rhn")
                        nc.vector.scalar_tensor_tensor(
                            out=rhn[:], in0=ps_gh[:, 256 + ht * 128 + C0: 256 + ht * 128 + C1],
                            scalar=bhhn[:, ht:ht + 1], in1=rz_sb[:, ht * 128 + C0: ht * 128 + C1],
                            op0=mybir.AluOpType.add, op1=mybir.AluOpType.mult)
                        nsum = work.tile([128, CW], f32, name="nsum", tag="nsum")
                        nc.vector.scalar_tensor_tensor(
                            out=nsum[:], in0=ps_gh[:, ht * 128 + C0: ht * 128 + C1],
                            scalar=bihn[:, ht:ht + 1], in1=rhn[:],
                            op0=mybir.AluOpType.add, op1=mybir.AluOpType.add)
                        n_t = work.tile([128, CW], f32, name="n_t", tag="n_t")
                        nc.scalar.activation(n_t[:], nsum[:], mybir.ActivationFunctionType.Tanh)
                        hmn = work.tile([128, CW], f32, name="hmn", tag="hmn")
                        nc.vector.tensor_sub(hmn[:], hloc[ht][:, C0:C1], n_t[:])
                        zh = work.tile([128, CW], f32, name="zh", tag="zh")
                        nc.vector.tensor_mul(zh[:], rz_sb[:, 256 + ht * 128 + C0: 256 + ht * 128 + C1], hmn[:])
                        nc.vector.tensor_add(hnxt[ht][:, C0:C1], n_t[:], zh[:])
                        nc.vector.tensor_copy(hTb[ht][:, C0:C1], hnxt[ht][:, C0:C1])
                        if t < T - 1:
                            nc.sync.dma_start(out=cc_in[t][ht * 128:(ht + 1) * 128, C0:C1], in_=hTb[ht][:, C0:C1])

                if t < T - 1:
                    nc.gpsimd.collective_compute(
                        "AllReduce", mybir.AluOpType.add, replica_groups=groups,
                        ins=[cc_in[t][:]], outs=[cc_out[t][:]])
                    for ht in range(HT):
                        rs = work.tile([128, NLOC], bf16, name="rs", tag="rs")
                        nc.sync.dma_start(out=rs[:], in_=cc_out[t][ht * 128:(ht + 1) * 128, :])
                        nc.vector.tensor_sub(hTb[ht][:, NLOC:N], rs[:], hTb[ht][:, 0:NLOC])

            # ---------------- readout ----------------
            gT = [work.tile([128, 1], f32, name=f"gT{ht}", tag=f"gT{ht}") for ht in range(HT)]
            for ht in range(HT):
                nc.vector.reduce_sum(gT[ht][:], hbuf[T % 2][ht][:], axis=mybir.AxisListType.X)
                nc.sync.dma_start(out=gcc_in[ht * 128:(ht + 1) * 128, :], in_=gT[ht][:])
            nc.gpsimd.collective_compute(
                "AllReduce", mybir.AluOpType.add, replica_groups=groups,
                ins=[gcc_in[:]], outs=[gcc_out[:]])
            gs = [work.tile([128, 1], f32, name=f"gs{ht}", tag=f"gs{ht}") for ht in range(HT)]
            for ht in range(HT):
                nc.sync.dma_start(out=gs[ht][:], in_=gcc_out[ht * 128:(ht + 1) * 128, :])
            y1 = [work.tile([128, 1], f32, name=f"y1{ht}", tag=f"y1{ht}") for ht in range(HT)]
            for ht in range(HT):
                ps = psp.tile([128, 512], f32, name="ps", tag="ps")
                for kt in range(HT):
                    nc.tensor.matmul(ps[:, 0:1], W_ro1[:, kt * H + ht * 128: kt * H + (ht + 1) * 128], gs[kt][:], start=(kt == 0), stop=(kt == HT - 1))
                nc.scalar.activation(y1[ht][:], ps[:, 0:1], mybir.ActivationFunctionType.Relu, bias=rob1[:, ht:ht + 1])
            ps_q = psp.tile([128, 512], f32, name="ps", tag="ps")
            for kt in range(HT):
                nc.tensor.matmul(ps_q[0:A, 0:1], W_ro2[:, kt * A:(kt + 1) * A], y1[kt][:], start=(kt == 0), stop=(kt == HT - 1))
            q_sb = work.tile([A, 1], f32, name="q_sb", tag="q_sb")
            nc.scalar.activation(q_sb[:], ps_q[0:A, 0:1], mybir.ActivationFunctionType.Identity, bias=rob2[:])
            nc.sync.dma_start(out=q_out[:], in_=q_sb[:])

    nc.compile()
    return nc


def _in_maps(inputs):
    nf = np.asarray(inputs["node_features"], np.float32)
    adj = np.asarray(inputs["adjacency"])
    msg_W1 = np.asarray(inputs["msg_W1"], np.float32)
    gbih = np.asarray(inputs["gru_bih"], np.float32)
    gbhh = np.asarray(inputs["gru_bhh"], np.float32)

    def cols(v, nt):  # [nt*128] -> [128, nt] partition-major columns
        return np.ascontiguousarray(np.asarray(v, np.float32).reshape(nt, 128).T)

    def wsb(w, dt=np.float32):  # [K, M] -> [128, (K//128)*M]
        w = np.asarray(w, np.float32)
        K, M = w.shape
        return np.ascontiguousarray(
            np.concatenate([w[k * 128:(k + 1) * 128] for k in range(K // 128)], axis=1)
        ).astype(dt)

    shared = {
        "pre_W1": np.asarray(inputs["pre_W1"], np.float32),
        "pre_W2": wsb(inputs["pre_W2"]),
        "W1ib": wsb(msg_W1[:H], BF16_NP),
        "W1jb": wsb(msg_W1[H:], BF16_NP),
        "W2m": wsb(inputs["msg_W2"]),
        "Wih": wsb(inputs["gru_Wih"]),
        "Whh": wsb(inputs["gru_Whh"]),
        "roW1": wsb(inputs["ro_W1"]),
        "roW2": wsb(inputs["ro_W2"]),
        "preb1c": cols(inputs["pre_b1"], HT),
        "preb2c": cols(inputs["pre_b2"], HT),
        "msgb1c": cols(inputs["msg_b1"], HT),
        "msgb2r": np.asarray(inputs["msg_b2"], np.float32)[None, :],
        "brzc": cols((gbih + gbhh)[: 2 * H], 4),
        "bihnc": cols(gbih[2 * H:], HT),
        "bhhnc": cols(gbhh[2 * H:], HT),
        "rob1c": cols(inputs["ro_b1"], HT),
        "rob2c": np.asarray(inputs["ro_b2"], np.float32)[:, None],
        "identb": np.eye(128, dtype=np.float32).astype(BF16_NP),
        "onesr": np.ones((1, 128), np.float32).astype(BF16_NP),
    }
    maps = []
    for c in range(8):
        b, half = c // 2, c % 2
        lo, hi = half * NLOC, (half + 1) * NLOC
        perm = np.r_[lo:hi, 0:lo, hi:N]
        m = dict(shared)
        m["xT"] = np.ascontiguousarray(nf[b].T[:, perm])
        adjm = ((adj[b, lo:hi][:, perm] - 1) * 32).astype(np.float32)
        m["adjb"] = adjm.astype(BF16_NP)
        pe_rows = [cc * CW + r for cc in range(NCH) for r in range(MKPE)] or [0]
        m["adjpe"] = np.ascontiguousarray(adjm[pe_rows].reshape(1, -1)).astype(BF16_NP)
        m["degr"] = adj[b, lo:hi].sum(axis=1).astype(np.float32)[None, :]
        maps.append(m)
    return maps


def kernel(**inputs) -> np.ndarray:
    if "nc" not in _CACHE:
        _CACHE["nc"] = build_program()
    nc = _CACHE["nc"]
    maps = _in_maps(inputs)
    res = run_bass_kernel_spmd(nc, maps, list(range(8))).results
    q = np.stack([res[2 * b]["q_out"][:, 0] for b in range(B)]).astype(np.float32)
    return q
